# revision 33
# baseline (speedup 1.0000x reference)
"""Trainium2 Bass kernel for nn_DDM spline head.

reference:
  h = einsum('btd,tdo->bto', z, W) + b          # [B,120,672], 42-chunks per p=16
  gamma = h[...,0]; beta = softplus(h[...,1:22]); s = softmax(h[...,22:42]/0.1)
  out = [gamma, beta, concat(0, cumsum(s))]     # [B,120,16,43]

Strategy (pure data parallel over batch, 8 cores, B_loc=256 per core).
The cost model prices DMA purely by bytes (full rate once the contiguous
run is >=512B), so staging is dense (no 128-partition padding) and all
operands are f32r (single copy; rel tolerance 2e-2 leaves plenty of room):

  - Host: transpose z per core, append a ones row (folds bias b into the
    contraction), regroup W columns per t into [beta(336) | gamma(16) |
    10*s(320) | zeros(16)] (softmax 1/tau folded into the weights; the
    zero pad makes exp of the padded block a harmless 1.0), and merge z
    and W into ONE dense DRAM tensor zw [NG, 4, 17, 256+688] so each
    12-timestep chunk stages with just 4 DMAs (one per 32-partition row
    block k, for the tile_position=(32k,0) matmul row-tiling).
  - Device, per (block of 128 batch, 2 timesteps) PSUM tile [128,2,1024]
    (matmul slices stay inside 2KB PSUM banks: A=352 cols at 0, B=336
    cols at 512):
      2 f32r matmuls per t
      ACT: ONE fused exp over [beta | s-block] via a strided AP, writing
           eu/es at compact 336 pitch; ONE big Ln(eu+1) per 8-t block,
           emitted as the LAST writer of out_sb (whole-tile write
           tracking would otherwise stall it behind the delta writes)
      DVE: segmented-cumsum scan over es (mask reset per 20-segment; the
           16 pad cols are mask-dead), reciprocal of segment sums, gamma
           copy, a share of delta = C * (1/S)
      GPSIMD: the rest of the delta multiply + delta[0]=0 memsets
  - Output: two half-DMAs per block tile, issued right after the Ln.
"""

import numpy as np
from contextlib import ExitStack

import concourse.bass as bass
import concourse.bacc as bacc
import concourse.mybir as mybir
from concourse.tile import TileContext
from concourse.tile_rust import add_dep_helper
from concourse.bass_utils import run_bass_kernel_spmd

F32 = mybir.dt.float32
F32R = mybir.dt.float32r
AF = mybir.ActivationFunctionType
ALU = mybir.AluOpType

N_CORES = 8
B_FULL, T, D, P, CH = 2048, 120, 16, 16, 42
B_LOC = B_FULL // N_CORES          # 256
NBLK = B_LOC // 128                # 2
GP = 2                             # timesteps per PSUM tile
GS = 8                             # timesteps per block-iteration
HS = 4                             # timesteps per output half-DMA
NG = T // 4                        # 4-timestep groups (30)
CG = 3                             # groups per staged chunk (12 t)
NCHUNK = NG // CG                  # 10
N_A = 352                          # beta(336) + gamma(16)
N_B = 336                          # 10*s(320) + zeros(16)
NW = N_A + N_B                     # 688
NZW = B_LOC + NW                   # 944
# how many of the HS delta-multiply sub-units run on DVE (rest on GPSIMD)
MULT_DVE_SHARE = 2


def _round_f32r(a: np.ndarray) -> np.ndarray:
    u = a.view(np.uint32).astype(np.uint64)
    r = ((u + 0x800 + ((u >> 12) & 1)) & ~np.uint64(0xFFF)).astype(np.uint32)
    return r.view(np.float32)


def _pin_act_tables():
    """Make natural_log_exp_and_others the only set advertising Exp/Ln so
    bacc's table-load pass picks one set for both (else it alternates
    exp_and_others / natural_log and emits ~240 table loads)."""
    import concourse.bacc as _bacc
    real = _bacc.get_activation_tables

    def patched(arch):
        keep = "natural_log_exp_and_others"
        return {
            name: (fns if name == keep else fns - {AF.Exp, AF.Ln})
            for name, fns in real(arch).items()
        }

    _bacc.get_activation_tables = patched
    return real


def _build_nc():
    nc = bacc.Bacc()
    zw_d = nc.declare_dram_parameter("zw", [NG, 4, 17, NZW], F32R, isOutput=False)
    out_d = nc.declare_dram_parameter("out", [B_LOC, T, P, 43], F32, isOutput=True)

    with TileContext(nc) as tc, ExitStack() as ctx:
        zw_pool = ctx.enter_context(tc.tile_pool(name="zw", bufs=4))
        ee_pool = ctx.enter_context(tc.tile_pool(name="ee", bufs=2))
        c_pool = ctx.enter_context(tc.tile_pool(name="cc", bufs=2))
        r_pool = ctx.enter_context(tc.tile_pool(name="rr", bufs=2))
        out_pool = ctx.enter_context(tc.tile_pool(name="ob", bufs=3))
        const_pool = ctx.enter_context(tc.tile_pool(name="const", bufs=1))
        ps = ctx.enter_context(tc.tile_pool(name="ps", bufs=2, space="PSUM"))

        # scan mask over one es half ([HS, 336] at compact 336 pitch):
        # 1.0 everywhere, 0.0 at the start of each 20-segment
        mask = const_pool.tile([128, HS * 336], F32)
        nc.vector.memset(mask, 1.0)
        mask_z = bass.AP(
            tensor=mask.tensor, offset=mask.offset,
            ap=[mask.ap[0], [336, HS], [20, P], [1, 1]],
        )
        nc.vector.memset(mask_z, 0.0)

        def do_stage(c):
            # dense staging: per k in 0..4, one DMA into partition rows
            # 32k..32k+17 (contiguous runs ~3.7KB -> full model bandwidth)
            zw_t = zw_pool.tile([128, CG, NZW], F32R, tag="zw")
            for k in range(4):
                rows = slice(32 * k, 32 * k + 17)
                nc.sync.dma_start(
                    out=zw_t[rows],
                    in_=zw_d[c * CG:(c + 1) * CG, k].rearrange("g r n -> r g n"))
            return zw_t

        staged = {c: do_stage(c) for c in range(3)}
        next_stage = 3

        def emit_u(cur, u):
            # matmuls + fused exp + gamma copy for one GP-timestep group
            bs, out_sb, ee, C, r, t0 = cur
            hp = ps.tile([128, GP, 1024], F32)
            for v in range(GP):
                t = t0 + u * GP + v
                c, local = t // (4 * CG), t % (4 * CG)
                gi, k4 = local // 4, local % 4
                zw_t = staged[c]
                rows = slice(32 * k4, 32 * k4 + 17)
                tp = (32 * k4, 0)
                nc.tensor.matmul(
                    hp[:, v, 0:N_A],
                    zw_t[rows, gi, bs],
                    zw_t[rows, gi, B_LOC:B_LOC + N_A],
                    start=True, stop=True, tile_position=tp,
                )
                nc.tensor.matmul(
                    hp[:, v, 512:512 + N_B],
                    zw_t[rows, gi, bs],
                    zw_t[rows, gi, B_LOC + N_A:],
                    start=True, stop=True, tile_position=tp,
                )
            # fused exp over [beta(336) | s-block(336)] per t:
            # eu_t -> ee[(u*GP+v)*336], es_t -> ee[GS*336 + ...]
            h_in = bass.AP(
                tensor=hp.tensor, offset=hp.offset,
                ap=[hp.ap[0], [1024, GP], [512, 2], [1, 336]],
            )
            ee_out = bass.AP(
                tensor=ee.tensor, offset=ee.offset + u * GP * 336,
                ap=[ee.ap[0], [336, GP], [GS * 336, 2], [1, 336]],
            )
            nc.scalar.activation(ee_out, h_in, AF.Exp)
            # gamma copy (PSUM cols 336:352)
            gamma_out = bass.AP(
                tensor=out_sb.tensor,
                offset=out_sb.offset + u * GP * P * 43,
                ap=[out_sb.ap[0], [P * 43, GP], [43, P], [1, 1]],
            )
            nc.vector.tensor_copy(
                gamma_out,
                hp[:, :, 336:352].rearrange("q a (b c) -> q a b c", c=1),
            )

        def tail_half(cur, half):
            # segmented inclusive cumsum over this half's es region (junk
            # cols between segments hold exp(0)=1, mask-dead), then
            # reciprocal of segment sums and delta = C * (1/S)
            bs, out_sb, ee, C, r, t0 = cur
            if half == 0:
                nc.vector.memset(C[:, 0:1], 0.0)
            h0 = half * HS * 336
            nc.vector.tensor_tensor_scan(
                C[:, 1 + h0:1 + h0 + HS * 336], mask,
                ee[:, GS * 336 + h0:GS * 336 + h0 + HS * 336],
                0.0, ALU.mult, ALU.add,
            )
            S_ap = bass.AP(
                tensor=C.tensor, offset=C.offset + 20 + h0,
                ap=[C.ap[0], [336, HS], [20, P]],
            )
            r_v = bass.AP(
                tensor=r.tensor, offset=r.offset + half * HS * P,
                ap=[r.ap[0], [P, HS], [1, P]],
            )
            recip_i = nc.vector.reciprocal(r_v, S_ap)
            for eng, lo, hi in (
                (nc.vector, 0, MULT_DVE_SHARE),
                (nc.gpsimd, MULT_DVE_SHARE, HS),
            ):
                if lo >= hi:
                    continue
                n = hi - lo
                delta_out = bass.AP(
                    tensor=out_sb.tensor,
                    offset=out_sb.offset + (half * HS + lo) * P * 43 + 22,
                    ap=[out_sb.ap[0], [P * 43, n], [43, P], [1, 21]],
                )
                C_in = bass.AP(
                    tensor=C.tensor, offset=C.offset + h0 + lo * 336,
                    ap=[C.ap[0], [336, n], [20, P], [1, 21]],
                )
                r_b = bass.AP(
                    tensor=r.tensor,
                    offset=r.offset + (half * HS + lo) * P,
                    ap=[r.ap[0], [P, n], [1, P], [0, 21]],
                )
                mult_i = eng.tensor_tensor(delta_out, C_in, r_b, ALU.mult)
                # the 0-step broadcast dim on r defeats subtile RAW
                # tracking; enforce recip -> mult explicitly
                add_dep_helper(
                    mult_i.ins, recip_i.ins,
                    reason="delta mult reads r (0-step broadcast AP)",
                )
            # delta[0] = 0
            dz = bass.AP(
                tensor=out_sb.tensor,
                offset=out_sb.offset + half * HS * P * 43 + 22,
                ap=[out_sb.ap[0], [P * 43, HS], [43, P], [1, 1]],
            )
            nc.gpsimd.memset(dz, 0.0)

        def tail_finish(cur):
            # h1 scan/delta, then the big Ln(eu + 1) -> beta columns (last
            # writer of out_sb in program order, no false WAW stall) and
            # the two output half-DMAs
            bs, out_sb, ee, C, r, t0 = cur
            tail_half(cur, 1)
            beta_out = bass.AP(
                tensor=out_sb.tensor, offset=out_sb.offset + 1,
                ap=[out_sb.ap[0], [P * 43, GS], [43, P], [1, 21]],
            )
            eu_v = bass.AP(
                tensor=ee.tensor, offset=ee.offset,
                ap=[ee.ap[0], [336, GS], [21, P], [1, 21]],
            )
            nc.scalar.activation(beta_out, eu_v, AF.Ln, bias=1.0)
            for half in range(2):
                nc.sync.dma_start(
                    out=out_d[bs, t0 + half * HS:t0 + (half + 1) * HS],
                    in_=out_sb[:, half * HS * P * 43:
                               (half + 1) * HS * P * 43].rearrange(
                        "q (a b c) -> q a b c", a=HS, b=P),
                )

        # software pipeline: each block's h1-tail + Ln + DMAs are emitted
        # in the MIDDLE of the next block's u-loop, so the next block's
        # gamma copies precede the long scan ops in DVE queue order and
        # PSUM buffers recycle without cross-engine stalls
        prev = None
        NITER = T // GS
        for it in range(NITER):
            t0 = it * GS
            for blk in range(NBLK):
                bs = slice(blk * 128, (blk + 1) * 128)
                out_sb = out_pool.tile([128, GS * P * 43], F32)
                # eu at [0, GS*336), es at [GS*336, 2*GS*336), both 336-pitch
                ee = ee_pool.tile([128, 2 * GS * 336], F32)
                C = c_pool.tile([128, 1 + GS * 336], F32)
                r = r_pool.tile([128, GS * P], F32)
                cur = (bs, out_sb, ee, C, r, t0)
                emit_u(cur, 0)
                emit_u(cur, 1)
                if prev is not None:
                    tail_finish(prev)
                emit_u(cur, 2)
                emit_u(cur, 3)
                tail_half(cur, 0)
                prev = cur
            # prefetch: keep every chunk iter i+2 touches staged; with
            # bufs=4 the overwritten buffer died ~3 iters ago, so these
            # DMAs never wait at the queue head in front of output DMAs
            need = min(NCHUNK - 1, (8 * (it + 2) + 7) // (4 * CG))
            while next_stage <= need:
                staged[next_stage] = do_stage(next_stage)
                next_stage += 1
        tail_finish(prev)
    real = _pin_act_tables()
    try:
        nc.compile()
    finally:
        import concourse.bacc as _bacc
        _bacc.get_activation_tables = real
    return nc


_NC = None


def prepare_in_maps(z, W, b):
    z = np.ascontiguousarray(z, dtype=np.float32)
    W = np.ascontiguousarray(W, dtype=np.float32)
    b = np.ascontiguousarray(b, dtype=np.float32)

    # regroup W columns per t: [beta(336) | gamma(16) | 10*s(320) | 0(16)],
    # bias as row 16 of each 17-row block
    W4 = W.reshape(T, D, P, CH)
    b3 = b.reshape(T, P, CH)
    w_beta = W4[:, :, :, 1:22].reshape(T, D, P * 21)
    b_beta = b3[:, :, 1:22].reshape(T, P * 21)
    w_g = W4[:, :, :, 0]                                  # [T,D,P]
    b_g = b3[:, :, 0]                                     # [T,P]
    w_s = 10.0 * W4[:, :, :, 22:].reshape(T, D, P * 20)
    b_s = 10.0 * b3[:, :, 22:].reshape(T, P * 20)
    zpad_w = np.zeros((T, D, P), dtype=np.float32)
    zpad_b = np.zeros((T, P), dtype=np.float32)
    w_all = np.concatenate([w_beta, w_g, w_s, zpad_w], axis=2)    # [T,D,688]
    b_all = np.concatenate([b_beta, b_g, b_s, zpad_b], axis=1)    # [T,688]
    w17 = _round_f32r(np.ascontiguousarray(
        np.concatenate([w_all, b_all[:, None, :]], axis=1)))      # [T,17,688]

    ones = np.ones((T, 1, B_LOC), dtype=np.float32)
    in_maps = []
    for c in range(N_CORES):
        zc = z[c * B_LOC:(c + 1) * B_LOC]                 # [256,120,16]
        zt = np.ascontiguousarray(zc.transpose(1, 2, 0))  # [120,16,256]
        zt17 = _round_f32r(np.ascontiguousarray(
            np.concatenate([zt, ones], axis=1)))          # [120,17,256]
        zw = np.concatenate([zt17, w17], axis=2)          # [120,17,944]
        in_maps.append({"zw": zw.reshape(NG, 4, 17, NZW)})
    return in_maps


def kernel(z: np.ndarray, W: np.ndarray, b: np.ndarray) -> np.ndarray:
    global _NC
    in_maps = prepare_in_maps(z, W, b)
    if _NC is None:
        _NC = _build_nc()
    res = run_bass_kernel_spmd(_NC, in_maps, list(range(N_CORES)))
    out = np.concatenate([r["out"] for r in res.results], axis=0)
    return out.astype(np.float32)


if __name__ == "__main__":
    rng = np.random.default_rng(0)
    z = rng.standard_normal((B_FULL, T, D)).astype(np.float32)
    W = (rng.standard_normal((T, D, P * CH)) * 0.05).astype(np.float32)
    b = (rng.standard_normal((T, P * CH)) * 0.05).astype(np.float32)
    out = kernel(z, W, b)
    print(out.shape, out.dtype)


# revision 37
# speedup vs baseline: 1.1028x; 1.1028x over previous
"""Trainium2 Bass kernel for nn_DDM spline head.

reference:
  h = einsum('btd,tdo->bto', z, W) + b          # [B,120,672], 42-chunks per p=16
  gamma = h[...,0]; beta = softplus(h[...,1:22]); s = softmax(h[...,22:42]/0.1)
  out = [gamma, beta, concat(0, cumsum(s))]     # [B,120,16,43]

Strategy (pure data parallel over batch, 8 cores, B_loc=256 per core).
The cost model prices DMA purely by bytes (full rate once the contiguous
run is >=512B), so staging is dense (no 128-partition padding) and all
operands are f32r (single copy; rel tolerance 2e-2 leaves plenty of room):

  - Host: transpose z per core, append a ones row (folds bias b into the
    contraction), regroup W columns per t into [beta(336) | gamma(16) |
    10*s(320) | zeros(16)] (softmax 1/tau folded into the weights; the
    zero pad makes exp of the padded block a harmless 1.0), and merge z
    and W into ONE dense DRAM tensor zw [NG, 4, 17, 256+688] so each
    12-timestep chunk stages with just 4 DMAs (one per 32-partition row
    block k, for the tile_position=(32k,0) matmul row-tiling).
  - Device, per (block of 128 batch, 2 timesteps) PSUM tile [128,2,1024]
    (matmul slices stay inside 2KB PSUM banks: A=352 cols at 0, B=336
    cols at 512):
      2 f32r matmuls per t
      ACT: ONE fused exp over [beta | s-block] via a strided AP, writing
           eu/es at compact 336 pitch; ONE big Ln(eu+1) per 8-t block,
           emitted as the LAST writer of out_sb (whole-tile write
           tracking would otherwise stall it behind the delta writes)
      DVE: segmented-cumsum scan over es (mask reset per 20-segment; the
           16 pad cols are mask-dead), reciprocal of segment sums, gamma
           copy, a share of delta = C * (1/S)
      GPSIMD: the rest of the delta multiply + delta[0]=0 memsets
  - Output: two half-DMAs per block tile, issued right after the Ln.
"""

import numpy as np
from contextlib import ExitStack

import concourse.bass as bass
import concourse.bacc as bacc
import concourse.mybir as mybir
from concourse.tile import TileContext
from concourse.tile_rust import add_dep_helper
from concourse.bass_utils import run_bass_kernel_spmd

F32 = mybir.dt.float32
F32R = mybir.dt.float32r
AF = mybir.ActivationFunctionType
ALU = mybir.AluOpType

N_CORES = 8
B_FULL, T, D, P, CH = 2048, 120, 16, 16, 42
B_LOC = B_FULL // N_CORES          # 256
NBLK = B_LOC // 128                # 2
GP = 2                             # timesteps per PSUM tile
GS = 8                             # timesteps per block-iteration
HS = 4                             # timesteps per output half-DMA
NG = T // 4                        # 4-timestep groups (30)
CG = 3                             # groups per staged chunk (12 t)
NCHUNK = NG // CG                  # 10
N_A = 352                          # beta(336) + gamma(16)
N_B = 336                          # 10*s(320) + zeros(16)
NW = N_A + N_B                     # 688
NZW = B_LOC + NW                   # 944
# how many of the HS delta-multiply sub-units run on DVE (rest on GPSIMD)
MULT_DVE_SHARE = 2


def _round_f32r(a: np.ndarray) -> np.ndarray:
    u = a.view(np.uint32).astype(np.uint64)
    r = ((u + 0x800 + ((u >> 12) & 1)) & ~np.uint64(0xFFF)).astype(np.uint32)
    return r.view(np.float32)


def _pin_act_tables():
    """Make natural_log_exp_and_others the only set advertising Exp/Ln so
    bacc's table-load pass picks one set for both (else it alternates
    exp_and_others / natural_log and emits ~240 table loads)."""
    import concourse.bacc as _bacc
    real = _bacc.get_activation_tables

    def patched(arch):
        keep = "natural_log_exp_and_others"
        return {
            name: (fns if name == keep else fns - {AF.Exp, AF.Ln})
            for name, fns in real(arch).items()
        }

    _bacc.get_activation_tables = patched
    return real


def _build_nc():
    nc = bacc.Bacc()
    zw_d = nc.declare_dram_parameter("zw", [NG, 4, 17, NZW], F32R, isOutput=False)
    out_d = nc.declare_dram_parameter("out", [B_LOC, T, P, 43], F32, isOutput=True)

    with TileContext(nc) as tc, ExitStack() as ctx:
        zw_pool = ctx.enter_context(tc.tile_pool(name="zw", bufs=4))
        ee_pool = ctx.enter_context(tc.tile_pool(name="ee", bufs=2))
        c_pool = ctx.enter_context(tc.tile_pool(name="cc", bufs=2))
        r_pool = ctx.enter_context(tc.tile_pool(name="rr", bufs=2))
        out_pool = ctx.enter_context(tc.tile_pool(name="ob", bufs=3))
        const_pool = ctx.enter_context(tc.tile_pool(name="const", bufs=1))
        ps = ctx.enter_context(tc.tile_pool(name="ps", bufs=2, space="PSUM"))

        # scan mask over one es half ([HS, 336] at compact 336 pitch):
        # 1.0 everywhere, 0.0 at the start of each 20-segment
        mask = const_pool.tile([128, HS * 336], F32)
        nc.vector.memset(mask, 1.0)
        mask_z = bass.AP(
            tensor=mask.tensor, offset=mask.offset,
            ap=[mask.ap[0], [336, HS], [20, P], [1, 1]],
        )
        nc.vector.memset(mask_z, 0.0)

        def do_stage(c):
            # dense staging: per k in 0..4, one DMA into partition rows
            # 32k..32k+17 (contiguous runs ~3.7KB -> full model bandwidth)
            zw_t = zw_pool.tile([128, CG, NZW], F32R, tag="zw")
            for k in range(4):
                rows = slice(32 * k, 32 * k + 17)
                nc.sync.dma_start(
                    out=zw_t[rows],
                    in_=zw_d[c * CG:(c + 1) * CG, k].rearrange("g r n -> r g n"))
            return zw_t

        staged = {c: do_stage(c) for c in range(3)}
        next_stage = 3

        def emit_u(cur, u):
            # matmuls + fused exp + gamma copy for one GP-timestep group
            bs, out_sb, ee, C, r, t0 = cur
            hp = ps.tile([128, GP, 1024], F32)
            for v in range(GP):
                t = t0 + u * GP + v
                c, local = t // (4 * CG), t % (4 * CG)
                gi, k4 = local // 4, local % 4
                zw_t = staged[c]
                rows = slice(32 * k4, 32 * k4 + 17)
                tp = (32 * k4, 0)
                nc.tensor.matmul(
                    hp[:, v, 0:N_A],
                    zw_t[rows, gi, bs],
                    zw_t[rows, gi, B_LOC:B_LOC + N_A],
                    start=True, stop=True, tile_position=tp,
                )
                nc.tensor.matmul(
                    hp[:, v, 512:512 + N_B],
                    zw_t[rows, gi, bs],
                    zw_t[rows, gi, B_LOC + N_A:],
                    start=True, stop=True, tile_position=tp,
                )
            # fused exp over [beta(336) | s-block(336)] per t:
            # eu_t -> ee[(u*GP+v)*336], es_t -> ee[GS*336 + ...]
            h_in = bass.AP(
                tensor=hp.tensor, offset=hp.offset,
                ap=[hp.ap[0], [1024, GP], [512, 2], [1, 336]],
            )
            ee_out = bass.AP(
                tensor=ee.tensor, offset=ee.offset + u * GP * 336,
                ap=[ee.ap[0], [336, GP], [GS * 336, 2], [1, 336]],
            )
            nc.scalar.activation(ee_out, h_in, AF.Exp)
            # gamma copy (PSUM cols 336:352)
            gamma_out = bass.AP(
                tensor=out_sb.tensor,
                offset=out_sb.offset + u * GP * P * 43,
                ap=[out_sb.ap[0], [P * 43, GP], [43, P], [1, 1]],
            )
            return nc.vector.tensor_copy(
                gamma_out,
                hp[:, :, 336:352].rearrange("q a (b c) -> q a b c", c=1),
            )

        def tail_half(cur, half, after=()):
            # segmented inclusive cumsum over this half's es region (junk
            # cols between segments hold exp(0)=1, mask-dead), then
            # reciprocal of segment sums and delta = C * (1/S)
            bs, out_sb, ee, C, r, t0 = cur
            if half == 0:
                nc.vector.memset(C[:, 0:1], 0.0)
            h0 = half * HS * 336
            scan_i = nc.vector.tensor_tensor_scan(
                C[:, 1 + h0:1 + h0 + HS * 336], mask,
                ee[:, GS * 336 + h0:GS * 336 + h0 + HS * 336],
                0.0, ALU.mult, ALU.add,
            )
            for g in after:
                # force the tiny PSUM-releasing gamma copies ahead of this
                # long scan in DVE queue order (Tile otherwise hoists the
                # scan, stalling the next matmuls on PSUM recycling)
                add_dep_helper(
                    scan_i.ins, g.ins,
                    reason="scan yields DVE to PSUM-releasing gamma copies",
                )
            S_ap = bass.AP(
                tensor=C.tensor, offset=C.offset + 20 + h0,
                ap=[C.ap[0], [336, HS], [20, P]],
            )
            r_v = bass.AP(
                tensor=r.tensor, offset=r.offset + half * HS * P,
                ap=[r.ap[0], [P, HS], [1, P]],
            )
            recip_i = nc.vector.reciprocal(r_v, S_ap)
            for eng, lo, hi in (
                (nc.vector, 0, MULT_DVE_SHARE),
                (nc.gpsimd, MULT_DVE_SHARE, HS),
            ):
                if lo >= hi:
                    continue
                n = hi - lo
                delta_out = bass.AP(
                    tensor=out_sb.tensor,
                    offset=out_sb.offset + (half * HS + lo) * P * 43 + 22,
                    ap=[out_sb.ap[0], [P * 43, n], [43, P], [1, 21]],
                )
                C_in = bass.AP(
                    tensor=C.tensor, offset=C.offset + h0 + lo * 336,
                    ap=[C.ap[0], [336, n], [20, P], [1, 21]],
                )
                r_b = bass.AP(
                    tensor=r.tensor,
                    offset=r.offset + (half * HS + lo) * P,
                    ap=[r.ap[0], [P, n], [1, P], [0, 21]],
                )
                mult_i = eng.tensor_tensor(delta_out, C_in, r_b, ALU.mult)
                # the 0-step broadcast dim on r defeats subtile RAW
                # tracking; enforce recip -> mult explicitly
                add_dep_helper(
                    mult_i.ins, recip_i.ins,
                    reason="delta mult reads r (0-step broadcast AP)",
                )
            # delta[0] = 0
            dz = bass.AP(
                tensor=out_sb.tensor,
                offset=out_sb.offset + half * HS * P * 43 + 22,
                ap=[out_sb.ap[0], [P * 43, HS], [43, P], [1, 1]],
            )
            nc.gpsimd.memset(dz, 0.0)

        def tail_finish(cur, after=()):
            # h1 scan/delta, then the big Ln(eu + 1) -> beta columns (last
            # writer of out_sb in program order, no false WAW stall) and
            # the two output half-DMAs
            bs, out_sb, ee, C, r, t0 = cur
            tail_half(cur, 1, after=after)
            beta_out = bass.AP(
                tensor=out_sb.tensor, offset=out_sb.offset + 1,
                ap=[out_sb.ap[0], [P * 43, GS], [43, P], [1, 21]],
            )
            eu_v = bass.AP(
                tensor=ee.tensor, offset=ee.offset,
                ap=[ee.ap[0], [336, GS], [21, P], [1, 21]],
            )
            nc.scalar.activation(beta_out, eu_v, AF.Ln, bias=1.0)
            for half in range(2):
                nc.sync.dma_start(
                    out=out_d[bs, t0 + half * HS:t0 + (half + 1) * HS],
                    in_=out_sb[:, half * HS * P * 43:
                               (half + 1) * HS * P * 43].rearrange(
                        "q (a b c) -> q a b c", a=HS, b=P),
                )

        # software pipeline: each block's h1-tail + Ln + DMAs are emitted
        # in the MIDDLE of the next block's u-loop, so the next block's
        # gamma copies precede the long scan ops in DVE queue order and
        # PSUM buffers recycle without cross-engine stalls
        prev = None
        NITER = T // GS
        for it in range(NITER):
            t0 = it * GS
            for blk in range(NBLK):
                bs = slice(blk * 128, (blk + 1) * 128)
                out_sb = out_pool.tile([128, GS * P * 43], F32)
                # eu at [0, GS*336), es at [GS*336, 2*GS*336), both 336-pitch
                ee = ee_pool.tile([128, 2 * GS * 336], F32)
                C = c_pool.tile([128, 1 + GS * 336], F32)
                r = r_pool.tile([128, GS * P], F32)
                cur = (bs, out_sb, ee, C, r, t0)
                g0 = emit_u(cur, 0)
                g1 = emit_u(cur, 1)
                if prev is not None:
                    tail_finish(prev, after=(g0, g1))
                g2 = emit_u(cur, 2)
                g3 = emit_u(cur, 3)
                tail_half(cur, 0, after=(g2, g3))
                prev = cur
            # prefetch: keep every chunk iter i+2 touches staged; with
            # bufs=4 the overwritten buffer died ~3 iters ago, so these
            # DMAs never wait at the queue head in front of output DMAs
            need = min(NCHUNK - 1, (8 * (it + 2) + 7) // (4 * CG))
            while next_stage <= need:
                staged[next_stage] = do_stage(next_stage)
                next_stage += 1
        tail_finish(prev)
    real = _pin_act_tables()
    try:
        nc.compile()
    finally:
        import concourse.bacc as _bacc
        _bacc.get_activation_tables = real
    return nc


_NC = None


def prepare_in_maps(z, W, b):
    z = np.ascontiguousarray(z, dtype=np.float32)
    W = np.ascontiguousarray(W, dtype=np.float32)
    b = np.ascontiguousarray(b, dtype=np.float32)

    # regroup W columns per t: [beta(336) | gamma(16) | 10*s(320) | 0(16)],
    # bias as row 16 of each 17-row block
    W4 = W.reshape(T, D, P, CH)
    b3 = b.reshape(T, P, CH)
    w_beta = W4[:, :, :, 1:22].reshape(T, D, P * 21)
    b_beta = b3[:, :, 1:22].reshape(T, P * 21)
    w_g = W4[:, :, :, 0]                                  # [T,D,P]
    b_g = b3[:, :, 0]                                     # [T,P]
    w_s = 10.0 * W4[:, :, :, 22:].reshape(T, D, P * 20)
    b_s = 10.0 * b3[:, :, 22:].reshape(T, P * 20)
    zpad_w = np.zeros((T, D, P), dtype=np.float32)
    zpad_b = np.zeros((T, P), dtype=np.float32)
    w_all = np.concatenate([w_beta, w_g, w_s, zpad_w], axis=2)    # [T,D,688]
    b_all = np.concatenate([b_beta, b_g, b_s, zpad_b], axis=1)    # [T,688]
    w17 = _round_f32r(np.ascontiguousarray(
        np.concatenate([w_all, b_all[:, None, :]], axis=1)))      # [T,17,688]

    ones = np.ones((T, 1, B_LOC), dtype=np.float32)
    in_maps = []
    for c in range(N_CORES):
        zc = z[c * B_LOC:(c + 1) * B_LOC]                 # [256,120,16]
        zt = np.ascontiguousarray(zc.transpose(1, 2, 0))  # [120,16,256]
        zt17 = _round_f32r(np.ascontiguousarray(
            np.concatenate([zt, ones], axis=1)))          # [120,17,256]
        zw = np.concatenate([zt17, w17], axis=2)          # [120,17,944]
        in_maps.append({"zw": zw.reshape(NG, 4, 17, NZW)})
    return in_maps


def kernel(z: np.ndarray, W: np.ndarray, b: np.ndarray) -> np.ndarray:
    global _NC
    in_maps = prepare_in_maps(z, W, b)
    if _NC is None:
        _NC = _build_nc()
    res = run_bass_kernel_spmd(_NC, in_maps, list(range(N_CORES)))
    out = np.concatenate([r["out"] for r in res.results], axis=0)
    return out.astype(np.float32)


if __name__ == "__main__":
    rng = np.random.default_rng(0)
    z = rng.standard_normal((B_FULL, T, D)).astype(np.float32)
    W = (rng.standard_normal((T, D, P * CH)) * 0.05).astype(np.float32)
    b = (rng.standard_normal((T, P * CH)) * 0.05).astype(np.float32)
    out = kernel(z, W, b)
    print(out.shape, out.dtype)
